# revision 17
# baseline (speedup 1.0000x reference)
"""Trainium2 Bass kernel for nn_DependencyParsing (embedding_lookup).

Strategy (pure data-parallel over 8 NeuronCores, B=65536 -> 8192/core):
  - Everything on the PE runs as fp8(e4m3) DoubleRow matmuls: each
    instruction contracts 256 K-slots (128 partitions x 2 byte-planes)
    at the same per-instruction cost as bf16 (~226ns @ N=512).
  - word_table rows are 256B of packed fp8 feature-pairs (x16 scale):
    feature f of a row lands at SBUF (partition f//2, byte f%2) via the
    same u16 transpose dma_gather as a bf16 table would use. 4 SWDGE
    queues sustain ~1.27us/gather (112 gathers -> ~142us), overlapped
    with compute. Gathers also zero partitions 50..127 (table row tail).
  - pos/dep one-hots are precomputed on the host as fp8 pairs and DMA'd
    into partitions 50..98 of the *same* gathered tiles (after the
    gathers; HWDGE concurrent with SWDGE verified clean). The matching
    rows of the combined weight tensor hold host-computed
    proj = table_s @ W_s (x256) and the summed bias (x256) rides a
    constant-one slot at (t=0, partition 98). So h = x @ W is just
    7 DoubleRow matmuls per M-tile: 42 + 3 (logits) = 45 PE instrs per
    512-sample chunk.
  - h (PSUM, x256) -> ACT Square(scale 2^-8) -> DVE mul -> h3 fp8 (x256)
    packed [128, 3, 2, 512] for 3 DoubleRow logits matmuls (Wo x16).
  - ACT Exp(lg x 2^-12 + bo) -> ex bf16 [93, 512] -> DMA out per chunk.
    Softmax normalization (divide by row sum) happens on the host.
  - The previous chunk's logits/exp/out are interleaved between the
    current chunk's M-tiles so the PE never idles long enough for the
    HAM clock gate to re-throttle.
"""

import os

import numpy as np
import ml_dtypes

import concourse.bacc as bacc
import concourse.mybir as mybir
import concourse.tile as tile
from concourse.tile import add_dep_helper
from concourse.bass_utils import run_bass_kernel_spmd

B, T, D, H, V, NPOS, NDEP, OUT = 65536, 7, 100, 700, 32000, 50, 45, 93
NCORES = 8
B_CORE = B // NCORES
CHUNK = 512
P = 128
MT = [(0, 128), (128, 128), (256, 128), (384, 128), (512, 128), (640, 64)]
MOFF = [0, 256, 512, 768, 1024, 1280]  # 2*msz-prefix offsets within a t-block
TBLK = 1408  # 2 * sum(msz) per token
dt = mybir.dt
f8 = ml_dtypes.float8_e4m3
bf16 = ml_dtypes.bfloat16
NQ = int(os.environ.get("KERNEL_NQ", "4"))
DR = mybir.MatmulPerfMode.DoubleRow
F8_ONE = np.float32(1.0).astype(f8).view(np.uint8)  # 0x38

_NC_CACHE = {}


def build_nc(b_core):
    n_chunks = b_core // CHUNK
    nc = bacc.Bacc(None, target_bir_lowering=False, num_swdge_queues=NQ)
    with tile.TileContext(nc) as tc:
        with tc.tile_pool(name="dram", bufs=1, space="DRAM") as dram:
            word_tab = dram.tile([V + 1, 128], dt.uint16, kind="ExternalInput",
                                 name="word_tab", uniquify=False)
            widx_d = dram.tile([P, T * n_chunks * 32], dt.int16,
                               kind="ExternalInput", name="widx", uniquify=False)
            ohx_d = dram.tile([49, n_chunks * T * CHUNK], dt.uint16,
                              kind="ExternalInput", name="ohx", uniquify=False)
            wcomb_d = dram.tile([P, T * TBLK], dt.float8e4,
                                kind="ExternalInput", name="wcomb", uniquify=False)
            wo_d = dram.tile([P, 3 * 2 * 96], dt.float8e4,
                             kind="ExternalInput", name="w_o", uniquify=False)
            bo_d = dram.tile([P, 1], dt.float32, kind="ExternalInput",
                             name="bo_pad", uniquify=False)
            out_d = dram.tile([OUT, b_core], dt.bfloat16, kind="ExternalOutput",
                              name="out", uniquify=False)

            with (
                tc.tile_pool(name="const", bufs=1) as const,
                tc.tile_pool(name="wt", bufs=3) as wt_pool,
                tc.tile_pool(name="sq", bufs=3) as sq_pool,
                tc.tile_pool(name="h3", bufs=3) as h3_pool,
                tc.tile_pool(name="exq", bufs=3) as ex_pool,
                tc.tile_pool(name="hps", bufs=1, space="PSUM") as hps_pool,
                tc.tile_pool(name="ltps", bufs=2, space="PSUM") as ltps_pool,
            ):
                # widx is chunk-major and preloaded in per-chunk pieces so
                # chunk 0's gathers only wait on a 57KB DMA, not the whole
                # 0.9MB. HWDGE preloads run concurrently with SWDGE gathers
                # (verified clean on HW).
                # Preloads go through the ACT engine's HWDGE queue so the
                # Sync queue is free for the per-chunk one-hot/output DMAs.
                widx_sb = const.tile([P, n_chunks * T * 32], dt.int16,
                                     name="widx_sb")
                wseg = T * 32

                def widx_piece(c0, c1):
                    nc.scalar.dma_start(
                        out=widx_sb[:, c0 * wseg:c1 * wseg],
                        in_=widx_d[:, c0 * wseg:c1 * wseg])

                widx_piece(0, 1)
                widx_piece(1, 2)
                wcomb_sb = const.tile([P, T * TBLK], dt.float8e4, name="wcomb_sb")
                nc.scalar.dma_start(out=wcomb_sb[:], in_=wcomb_d[:])
                widx_piece(2, 4)
                wo_sb = const.tile([P, 3 * 2 * 96], dt.float8e4, name="wo_sb")
                nc.scalar.dma_start(out=wo_sb[:], in_=wo_d[:])
                bo_sb = const.tile([P, 1], dt.float32, name="bo_sb")
                nc.scalar.dma_start(out=bo_sb[:], in_=bo_d[:])
                widx_piece(4, n_chunks)

                wo_v = wo_sb.rearrange("p (r j m) -> p r j m", r=3, j=2)

                def wslice(t, mi, msz):
                    o = t * TBLK + MOFF[mi]
                    return wcomb_sb[:, o:o + 2 * msz].rearrange(
                        "p (j m) -> p j m", j=2)

                # zero the h3 tail (plane 5, partitions 60..127) once per
                # physical buffer: stale SBUF bytes there could decode as
                # fp8 NaN and poison the logits accumulation.
                h3_warm = []
                for _ in range(3):
                    hw = h3_pool.tile([P, 3, 2, CHUNK], dt.float8e4, name="h3")
                    nc.vector.memset(hw[:, 2, 1, :], 0.0)
                    h3_warm.append(hw)

                pend = {}

                def emit_logits_pair(h3_ref, pr):
                    if pr == 0:
                        pend["lg"] = ltps_pool.tile([P, CHUNK], dt.float32,
                                                    name="lg", tag="lt")
                    nc.tensor.matmul(
                        pend["lg"][:96, :], wo_v[:, pr, :, :],
                        h3_ref[:, pr, :, :],
                        start=(pr == 0), stop=(pr == 2), perf_mode=DR)

                def emit_ex():
                    ex = ex_pool.tile([96, CHUNK], dt.bfloat16, name="ex")
                    nc.scalar.activation(ex[:96, :], pend["lg"][:96, :],
                                         mybir.ActivationFunctionType.Exp,
                                         bias=bo_sb[:96, :], scale=1.0 / 4096)
                    pend["ex"] = ex

                def emit_out(cc):
                    nc.sync.dma_start(out=out_d[:, cc * CHUNK:(cc + 1) * CHUNK],
                                      in_=pend["ex"][:OUT, :])

                qn = 0
                prev_h3 = None
                for c in range(n_chunks):
                    wt = wt_pool.tile([P, T, CHUNK], dt.uint16, name="wt")
                    for t in range(T):
                        nc.gpsimd.dma_gather(
                            wt[:, t:t + 1, :], word_tab[:],
                            widx_sb[:, (c * T + t) * 32:
                                    (c * T + t + 1) * 32],
                            CHUNK, CHUNK, 128, transpose=True,
                            queue_num=qn % NQ,
                        )
                        qn += 1
                    # host one-hots (pos/dep/bias) overwrite partitions
                    # 50..98 of every token's gathered tile
                    nc.sync.dma_start(
                        out=wt[50:99, :, :].rearrange("p a b -> p (a b)"),
                        in_=ohx_d[:, c * T * CHUNK:(c + 1) * T * CHUNK])

                    h3t = h3_pool.tile([P, 3, 2, CHUNK], dt.float8e4, name="h3")
                    for mi, (m0, msz) in enumerate(MT):
                        hp = hps_pool.tile([P, CHUNK], dt.float32,
                                           name=f"hps{mi}")
                        for t in range(T):
                            rhs = wt[:, t, :].bitcast(dt.float8e4).rearrange(
                                "p (n j) -> p j n", j=2)
                            nc.tensor.matmul(
                                hp[:msz, :], wslice(t, mi, msz),
                                rhs, start=(t == 0), stop=(t == T - 1),
                                perf_mode=DR)
                        sq = sq_pool.tile([P, CHUNK], dt.float32, name="sq")
                        nc.scalar.activation(sq[:msz, :], hp[:msz, :],
                                             mybir.ActivationFunctionType.Square,
                                             scale=1.0 / 256)
                        nc.vector.tensor_mul(h3t[:msz, mi // 2, mi % 2, :],
                                             sq[:msz, :], hp[:msz, :])
                        if mi == 0 and prev_h3 is not None:
                            emit_logits_pair(prev_h3, 2)
                        elif mi == 1 and prev_h3 is not None:
                            emit_ex()
                        elif mi == 2 and prev_h3 is not None:
                            emit_out(c - 1)
                        elif mi == 3:
                            emit_logits_pair(h3t, 0)
                        elif mi == 4:
                            emit_logits_pair(h3t, 1)
                    prev_h3 = h3t

                emit_logits_pair(prev_h3, 2)
                emit_ex()
                emit_out(n_chunks - 1)
    nc.compile()
    return nc


def _wrap_idx(idx_tc):
    """[CHUNK] -> [128, 32] wrapped (i -> [i%16, i//16]) + replicated x8."""
    n = idx_tc.shape[0]
    w = idx_tc.reshape(n // 16, 16).T
    return np.tile(w, (8, 1))


def prep_inputs(word_idx, pos_idx, dep_idx, word_table, pos_table, dep_table,
                Ww, bw, Wp, bp, Wd, bd, Wo, bo, b_core):
    """Returns (shared_map, per_core_fn)."""
    n_chunks = b_core // CHUNK

    # ---- word table: 256B rows of fp8 pairs (x16) ----
    wt8 = np.zeros((V + 1, 256), dtype=np.uint8)
    wt8[:V, :D] = (np.asarray(word_table, np.float32) * 16).astype(f8).view(
        np.uint8)
    wt_q = wt8.view(np.uint16)  # [V+1, 128]

    # ---- combined weights [128, T, 2, H] fp8 ----
    Ww32 = np.asarray(Ww, np.float32)
    Wp32 = np.asarray(Wp, np.float32)
    Wd32 = np.asarray(Wd, np.float32)
    pt32 = np.asarray(pos_table, np.float32)
    dt32 = np.asarray(dep_table, np.float32)
    bias_all = (np.asarray(bw, np.float32) + np.asarray(bp, np.float32)
                + np.asarray(bd, np.float32))

    wfull = np.zeros((P, T, 2, H), dtype=f8)
    for t in range(T):
        wq = (Ww32[D * t:D * (t + 1)] * 16).astype(f8)        # [100, H]
        f = np.arange(D)
        wfull[f // 2, t, f % 2, :] = wq
        projp = (pt32 @ Wp32[D * t:D * (t + 1)] * 256).astype(f8)  # [50, H]
        s = np.arange(NPOS)
        wfull[50 + s // 2, t, s % 2, :] = projp
        projd = (dt32 @ Wd32[D * t:D * (t + 1)] * 256).astype(f8)  # [45, H]
        s = np.arange(NDEP)
        wfull[75 + s // 2, t, s % 2, :] = projd
    wfull[98, 0, 0, :] = (bias_all * 256).astype(f8)
    # repack into contiguous per-(t, M-tile) [2, msz] blocks
    wcomb_host = np.zeros((P, T * TBLK), dtype=f8)
    for t in range(T):
        for mi, (m0, msz) in enumerate(MT):
            mhi = min(m0 + msz, H)
            blk = np.zeros((P, 2, msz), dtype=f8)
            blk[:, :, :mhi - m0] = wfull[:, t, :, m0:mhi]
            o = t * TBLK + MOFF[mi]
            wcomb_host[:, o:o + 2 * msz] = blk.reshape(P, 2 * msz)

    # ---- logits weights [128, 3, 2, 96] fp8: slot (p,pr,j) = h-feat
    # 128*(2pr+j)+p ----
    Wo32 = np.asarray(Wo, np.float32)
    wo = np.zeros((P, 3, 2, 96), dtype=f8)
    for pr in range(3):
        for j in range(2):
            mi = 2 * pr + j
            m0, msz = MT[mi]
            mhi = min(m0 + msz, H)
            wo[:mhi - m0, pr, j, :OUT] = (Wo32[m0:mhi] * 16).astype(f8)
    wo_host = np.ascontiguousarray(wo).reshape(P, 3 * 2 * 96)

    bo_pad = np.zeros((P, 1), dtype=np.float32)
    bo_pad[:OUT, 0] = np.asarray(bo, np.float32)

    shared = {
        "word_tab": wt_q,
        "wcomb": wcomb_host,
        "w_o": wo_host,
        "bo_pad": bo_pad,
    }

    wi = np.asarray(word_idx, np.int64).copy()
    wi[wi < 0] = V
    wi = wi.astype(np.int16)
    pi = np.asarray(pos_idx, np.int32)
    di = np.asarray(dep_idx, np.int32)

    def core_map(core):
        s = slice(core * b_core, (core + 1) * b_core)
        wic = wi[s]
        widx = np.zeros((P, n_chunks, T, 32), dtype=np.int16)
        for t in range(T):
            for c in range(n_chunks):
                widx[:, c, t, :] = _wrap_idx(wic[c * CHUNK:(c + 1) * CHUNK, t])

        # one-hot pairs: [49, 2, n_chunks, T, CHUNK] u8 -> u16 [49, ...]
        oh = np.zeros((49, 2, n_chunks, T, CHUNK), dtype=np.uint8)
        pc = pi[s].reshape(n_chunks, CHUNK, T)
        dc = di[s].reshape(n_chunks, CHUNK, T)
        cg, ig, tg = np.meshgrid(np.arange(n_chunks), np.arange(CHUNK),
                                 np.arange(T), indexing="ij")
        oh[pc // 2, pc % 2, cg, tg, ig] = F8_ONE
        oh[25 + dc // 2, dc % 2, cg, tg, ig] = F8_ONE
        oh[48, 0, :, 0, :] = F8_ONE  # bias rides token 0
        ohx = np.ascontiguousarray(oh.transpose(0, 2, 3, 4, 1)).view(
            np.uint16).reshape(49, n_chunks * T * CHUNK)

        m = dict(shared)
        m["widx"] = widx.reshape(P, T * n_chunks * 32)
        m["ohx"] = ohx
        return m

    return shared, core_map


def kernel(**inputs):
    b_core = B_CORE
    if b_core not in _NC_CACHE:
        _NC_CACHE[b_core] = build_nc(b_core)
    nc = _NC_CACHE[b_core]

    _, core_map = prep_inputs(b_core=b_core, **inputs)
    in_maps = [core_map(i) for i in range(NCORES)]
    res = run_bass_kernel_spmd(nc, in_maps, core_ids=list(range(NCORES)))
    ex = np.concatenate([r["out"] for r in res.results], axis=1)  # [93, B]
    ex = np.ascontiguousarray(ex.T).astype(np.float32)            # [B, 93]
    return ex / ex.sum(axis=1, keepdims=True)


# revision 19
# speedup vs baseline: 1.0839x; 1.0839x over previous
"""Trainium2 Bass kernel for nn_DependencyParsing (embedding_lookup).

Strategy (pure data-parallel over 8 NeuronCores, B=65536 -> 8192/core):
  - Everything on the PE runs as fp8(e4m3) DoubleRow matmuls: each
    instruction contracts 256 K-slots (128 partitions x 2 byte-planes)
    at the same per-instruction cost as bf16 (~226ns @ N=512).
  - word_table rows are 256B of packed fp8 feature-pairs (x16 scale):
    feature f of a row lands at SBUF (partition f//2, byte f%2) via the
    same u16 transpose dma_gather as a bf16 table would use. 4 SWDGE
    queues sustain ~1.27us/gather (112 gathers -> ~142us), overlapped
    with compute. Gathers also zero partitions 50..127 (table row tail).
  - pos/dep one-hots are precomputed on the host as fp8 pairs and DMA'd
    into partitions 50..98 of the *same* gathered tiles (after the
    gathers; HWDGE concurrent with SWDGE verified clean). The matching
    rows of the combined weight tensor hold host-computed
    proj = table_s @ W_s (x256) and the summed bias (x256) rides a
    constant-one slot at (t=0, partition 98). So h = x @ W is just
    7 DoubleRow matmuls per M-tile: 42 + 3 (logits) = 45 PE instrs per
    512-sample chunk.
  - h (PSUM, x256) -> ACT Square(scale 2^-8) -> DVE mul -> h3 fp8 (x256)
    packed [128, 3, 2, 512] for 3 DoubleRow logits matmuls (Wo x16).
  - ACT Exp(lg x 2^-12 + bo) -> ex bf16 [93, 512] -> DMA out per chunk.
    Softmax normalization (divide by row sum) happens on the host.
  - The previous chunk's logits/exp/out are interleaved between the
    current chunk's M-tiles so the PE never idles long enough for the
    HAM clock gate to re-throttle.
"""

import os

import numpy as np
import ml_dtypes

import concourse.bacc as bacc
import concourse.mybir as mybir
import concourse.tile as tile
from concourse.tile import add_dep_helper
from concourse.bass_utils import run_bass_kernel_spmd

B, T, D, H, V, NPOS, NDEP, OUT = 65536, 7, 100, 700, 32000, 50, 45, 93
NCORES = 8
B_CORE = B // NCORES
CHUNK = 512
P = 128
MT = [(0, 128), (128, 128), (256, 128), (384, 128), (512, 128), (640, 64)]
MOFF = [0, 256, 512, 768, 1024, 1280]  # 2*msz-prefix offsets within a t-block
TBLK = 1408  # 2 * sum(msz) per token
dt = mybir.dt
f8 = ml_dtypes.float8_e4m3
bf16 = ml_dtypes.bfloat16
NQ = int(os.environ.get("KERNEL_NQ", "4"))
DR = mybir.MatmulPerfMode.DoubleRow
F8_ONE = np.float32(1.0).astype(f8).view(np.uint8)  # 0x38

_NC_CACHE = {}


def build_nc(b_core):
    n_chunks = b_core // CHUNK
    nc = bacc.Bacc(None, target_bir_lowering=False, num_swdge_queues=NQ)
    with tile.TileContext(nc) as tc:
        with tc.tile_pool(name="dram", bufs=1, space="DRAM") as dram:
            word_tab = dram.tile([V + 1, 128], dt.uint16, kind="ExternalInput",
                                 name="word_tab", uniquify=False)
            widx_d = dram.tile([P, T * n_chunks * 32], dt.int16,
                               kind="ExternalInput", name="widx", uniquify=False)
            ohx_d = dram.tile([49, n_chunks * T * CHUNK], dt.uint16,
                              kind="ExternalInput", name="ohx", uniquify=False)
            wcomb_d = dram.tile([P, T * TBLK], dt.float8e4,
                                kind="ExternalInput", name="wcomb", uniquify=False)
            wo_d = dram.tile([P, 3 * 2 * 96], dt.float8e4,
                             kind="ExternalInput", name="w_o", uniquify=False)
            bo_d = dram.tile([P, 1], dt.float32, kind="ExternalInput",
                             name="bo_pad", uniquify=False)
            out_d = dram.tile([OUT, b_core], dt.bfloat16, kind="ExternalOutput",
                              name="out", uniquify=False)

            with (
                tc.tile_pool(name="const", bufs=1) as const,
                tc.tile_pool(name="wt", bufs=4) as wt_pool,
                tc.tile_pool(name="sq", bufs=3) as sq_pool,
                tc.tile_pool(name="h3", bufs=3) as h3_pool,
                tc.tile_pool(name="exq", bufs=3) as ex_pool,
                tc.tile_pool(name="hps", bufs=1, space="PSUM") as hps_pool,
                tc.tile_pool(name="ltps", bufs=2, space="PSUM") as ltps_pool,
            ):
                # widx is chunk-major and preloaded in per-chunk pieces so
                # chunk 0's gathers only wait on a 57KB DMA, not the whole
                # 0.9MB. HWDGE preloads run concurrently with SWDGE gathers
                # (verified clean on HW).
                # Preloads go through the ACT engine's HWDGE queue so the
                # Sync queue is free for the per-chunk one-hot/output DMAs.
                widx_sb = const.tile([P, n_chunks * T * 32], dt.int16,
                                     name="widx_sb")
                wseg = T * 32

                def widx_piece(c0, c1):
                    nc.scalar.dma_start(
                        out=widx_sb[:, c0 * wseg:c1 * wseg],
                        in_=widx_d[:, c0 * wseg:c1 * wseg])

                widx_piece(0, 1)
                widx_piece(1, 2)
                wcomb_sb = const.tile([P, T * TBLK], dt.float8e4, name="wcomb_sb")
                nc.scalar.dma_start(out=wcomb_sb[:], in_=wcomb_d[:])
                widx_piece(2, 4)
                wo_sb = const.tile([P, 3 * 2 * 96], dt.float8e4, name="wo_sb")
                nc.scalar.dma_start(out=wo_sb[:], in_=wo_d[:])
                bo_sb = const.tile([P, 1], dt.float32, name="bo_sb")
                nc.scalar.dma_start(out=bo_sb[:], in_=bo_d[:])
                widx_piece(4, n_chunks)

                wo_v = wo_sb.rearrange("p (r j m) -> p r j m", r=3, j=2)

                def wslice(t, mi, msz):
                    o = t * TBLK + MOFF[mi]
                    return wcomb_sb[:, o:o + 2 * msz].rearrange(
                        "p (j m) -> p j m", j=2)

                # zero the h3 tail (plane 5, partitions 60..127) once per
                # physical buffer: stale SBUF bytes there could decode as
                # fp8 NaN and poison the logits accumulation.
                h3_warm = []
                for _ in range(3):
                    hw = h3_pool.tile([P, 3, 2, CHUNK], dt.float8e4, name="h3")
                    nc.vector.memset(hw[:, 2, 1, :], 0.0)
                    h3_warm.append(hw)

                pend = {}

                def emit_logits_pair(h3_ref, pr):
                    if pr == 0:
                        pend["lg"] = ltps_pool.tile([P, CHUNK], dt.float32,
                                                    name="lg", tag="lt")
                    nc.tensor.matmul(
                        pend["lg"][:96, :], wo_v[:, pr, :, :],
                        h3_ref[:, pr, :, :],
                        start=(pr == 0), stop=(pr == 2), perf_mode=DR)

                def emit_ex():
                    ex = ex_pool.tile([96, CHUNK], dt.bfloat16, name="ex")
                    nc.scalar.activation(ex[:96, :], pend["lg"][:96, :],
                                         mybir.ActivationFunctionType.Exp,
                                         bias=bo_sb[:96, :], scale=1.0 / 4096)
                    pend["ex"] = ex

                def emit_out(cc):
                    nc.sync.dma_start(out=out_d[:, cc * CHUNK:(cc + 1) * CHUNK],
                                      in_=pend["ex"][:OUT, :])

                qn = 0
                prev_h3 = None
                for c in range(n_chunks):
                    wt = wt_pool.tile([P, T, CHUNK], dt.uint16, name="wt")
                    for t in range(T):
                        nc.gpsimd.dma_gather(
                            wt[:, t:t + 1, :], word_tab[:],
                            widx_sb[:, (c * T + t) * 32:
                                    (c * T + t + 1) * 32],
                            CHUNK, CHUNK, 128, transpose=True,
                            queue_num=qn % NQ,
                        )
                        qn += 1
                        # host one-hots (pos/dep/bias) overwrite partitions
                        # 50..98 of this token's gathered column. One DMA per
                        # token: a single-predecessor WAW chain (gather t ->
                        # onehot t -> matmul t) that the dependency tracker
                        # orders correctly; a chunk-wide DMA raced the other
                        # queues' gathers.
                        nc.sync.dma_start(
                            out=wt[50:99, t, :],
                            in_=ohx_d[:, (c * T + t) * CHUNK:
                                      (c * T + t + 1) * CHUNK])

                    h3t = h3_pool.tile([P, 3, 2, CHUNK], dt.float8e4, name="h3")
                    for mi, (m0, msz) in enumerate(MT):
                        hp = hps_pool.tile([P, CHUNK], dt.float32,
                                           name=f"hps{mi}")
                        for t in range(T):
                            rhs = wt[:, t, :].bitcast(dt.float8e4).rearrange(
                                "p (n j) -> p j n", j=2)
                            nc.tensor.matmul(
                                hp[:msz, :], wslice(t, mi, msz),
                                rhs, start=(t == 0), stop=(t == T - 1),
                                perf_mode=DR)
                        sq = sq_pool.tile([P, CHUNK], dt.float32, name="sq")
                        nc.scalar.activation(sq[:msz, :], hp[:msz, :],
                                             mybir.ActivationFunctionType.Square,
                                             scale=1.0 / 256)
                        nc.vector.tensor_mul(h3t[:msz, mi // 2, mi % 2, :],
                                             sq[:msz, :], hp[:msz, :])
                        if mi == 0 and prev_h3 is not None:
                            emit_logits_pair(prev_h3, 2)
                        elif mi == 1 and prev_h3 is not None:
                            emit_ex()
                        elif mi == 2 and prev_h3 is not None:
                            emit_out(c - 1)
                        elif mi == 3:
                            emit_logits_pair(h3t, 0)
                        elif mi == 4:
                            emit_logits_pair(h3t, 1)
                    prev_h3 = h3t

                emit_logits_pair(prev_h3, 2)
                emit_ex()
                emit_out(n_chunks - 1)
    nc.compile()
    return nc


def _wrap_idx(idx_tc):
    """[CHUNK] -> [128, 32] wrapped (i -> [i%16, i//16]) + replicated x8."""
    n = idx_tc.shape[0]
    w = idx_tc.reshape(n // 16, 16).T
    return np.tile(w, (8, 1))


def prep_inputs(word_idx, pos_idx, dep_idx, word_table, pos_table, dep_table,
                Ww, bw, Wp, bp, Wd, bd, Wo, bo, b_core):
    """Returns (shared_map, per_core_fn)."""
    n_chunks = b_core // CHUNK

    # ---- word table: 256B rows of fp8 pairs (x16) ----
    wt8 = np.zeros((V + 1, 256), dtype=np.uint8)
    wt8[:V, :D] = (np.asarray(word_table, np.float32) * 16).astype(f8).view(
        np.uint8)
    wt_q = wt8.view(np.uint16)  # [V+1, 128]

    # ---- combined weights [128, T, 2, H] fp8 ----
    Ww32 = np.asarray(Ww, np.float32)
    Wp32 = np.asarray(Wp, np.float32)
    Wd32 = np.asarray(Wd, np.float32)
    pt32 = np.asarray(pos_table, np.float32)
    dt32 = np.asarray(dep_table, np.float32)
    bias_all = (np.asarray(bw, np.float32) + np.asarray(bp, np.float32)
                + np.asarray(bd, np.float32))

    wfull = np.zeros((P, T, 2, H), dtype=f8)
    for t in range(T):
        wq = (Ww32[D * t:D * (t + 1)] * 16).astype(f8)        # [100, H]
        f = np.arange(D)
        wfull[f // 2, t, f % 2, :] = wq
        projp = (pt32 @ Wp32[D * t:D * (t + 1)] * 256).astype(f8)  # [50, H]
        s = np.arange(NPOS)
        wfull[50 + s // 2, t, s % 2, :] = projp
        projd = (dt32 @ Wd32[D * t:D * (t + 1)] * 256).astype(f8)  # [45, H]
        s = np.arange(NDEP)
        wfull[75 + s // 2, t, s % 2, :] = projd
    wfull[98, 0, 0, :] = (bias_all * 256).astype(f8)
    # repack into contiguous per-(t, M-tile) [2, msz] blocks
    wcomb_host = np.zeros((P, T * TBLK), dtype=f8)
    for t in range(T):
        for mi, (m0, msz) in enumerate(MT):
            mhi = min(m0 + msz, H)
            blk = np.zeros((P, 2, msz), dtype=f8)
            blk[:, :, :mhi - m0] = wfull[:, t, :, m0:mhi]
            o = t * TBLK + MOFF[mi]
            wcomb_host[:, o:o + 2 * msz] = blk.reshape(P, 2 * msz)

    # ---- logits weights [128, 3, 2, 96] fp8: slot (p,pr,j) = h-feat
    # 128*(2pr+j)+p ----
    Wo32 = np.asarray(Wo, np.float32)
    wo = np.zeros((P, 3, 2, 96), dtype=f8)
    for pr in range(3):
        for j in range(2):
            mi = 2 * pr + j
            m0, msz = MT[mi]
            mhi = min(m0 + msz, H)
            wo[:mhi - m0, pr, j, :OUT] = (Wo32[m0:mhi] * 16).astype(f8)
    wo_host = np.ascontiguousarray(wo).reshape(P, 3 * 2 * 96)

    bo_pad = np.zeros((P, 1), dtype=np.float32)
    bo_pad[:OUT, 0] = np.asarray(bo, np.float32)

    shared = {
        "word_tab": wt_q,
        "wcomb": wcomb_host,
        "w_o": wo_host,
        "bo_pad": bo_pad,
    }

    wi = np.asarray(word_idx, np.int64).copy()
    wi[wi < 0] = V
    wi = wi.astype(np.int16)
    pi = np.asarray(pos_idx, np.int32)
    di = np.asarray(dep_idx, np.int32)

    def core_map(core):
        s = slice(core * b_core, (core + 1) * b_core)
        wic = wi[s]
        widx = np.zeros((P, n_chunks, T, 32), dtype=np.int16)
        for t in range(T):
            for c in range(n_chunks):
                widx[:, c, t, :] = _wrap_idx(wic[c * CHUNK:(c + 1) * CHUNK, t])

        # one-hot pairs: [49, 2, n_chunks, T, CHUNK] u8 -> u16 [49, ...]
        oh = np.zeros((49, 2, n_chunks, T, CHUNK), dtype=np.uint8)
        pc = pi[s].reshape(n_chunks, CHUNK, T)
        dc = di[s].reshape(n_chunks, CHUNK, T)
        cg, ig, tg = np.meshgrid(np.arange(n_chunks), np.arange(CHUNK),
                                 np.arange(T), indexing="ij")
        oh[pc // 2, pc % 2, cg, tg, ig] = F8_ONE
        oh[25 + dc // 2, dc % 2, cg, tg, ig] = F8_ONE
        oh[48, 0, :, 0, :] = F8_ONE  # bias rides token 0
        ohx = np.ascontiguousarray(oh.transpose(0, 2, 3, 4, 1)).view(
            np.uint16).reshape(49, n_chunks * T * CHUNK)

        m = dict(shared)
        m["widx"] = widx.reshape(P, T * n_chunks * 32)
        m["ohx"] = ohx
        return m

    return shared, core_map


def kernel(**inputs):
    b_core = B_CORE
    if b_core not in _NC_CACHE:
        _NC_CACHE[b_core] = build_nc(b_core)
    nc = _NC_CACHE[b_core]

    _, core_map = prep_inputs(b_core=b_core, **inputs)
    in_maps = [core_map(i) for i in range(NCORES)]
    res = run_bass_kernel_spmd(nc, in_maps, core_ids=list(range(NCORES)))
    ex = np.concatenate([r["out"] for r in res.results], axis=1)  # [93, B]
    ex = np.ascontiguousarray(ex.T).astype(np.float32)            # [B, 93]
    return ex / ex.sum(axis=1, keepdims=True)
